# revision 45
# baseline (speedup 1.0000x reference)
"""Trainium2 Bass kernel for a DynamicConv decoder layer.

Computation (fairseq DynamicConvDecoderLayer, eval mode, normalize_after):
    h  = x @ w1.T + b1                       # [T,B,E] -> [T,B,C]
    w  = softmax((h @ ww.T + bw) per-head)   # dynamic conv weights [T,B,H,K]
    c  = causal banded aggregation of h with per-position weights
    h2 = c @ w2.T + b2
    out = LayerNorm(x + h2) * gamma + beta

Distribution: data-parallel over batch (B=16 -> 2 per core on 8 cores).

Per-core algorithm (tokens b-major, m = b*T + t), fp8 DoubleRow formulation:
  - All big GEMMs run as fp8e4m3 DoubleRow matmuls (2 k-tiles per pass at
    0.5 cycles/row). Precision is recovered with hi/lo splits: operands are
    stored as fp8(v) plus fp8(v - fp8(v)); each GEMM runs 2-3 compensation
    terms, which lands the whole layer at rel err ~1.5e-2 (gate 2e-2)
    measured against the fp32 reference.
  - Phase A: h1 = x @ w1.T with xT hi/lo pair-packed per 256-row E-chunk.
  - Phase B: conv logits from x directly with host-fused (ww @ w1)^T.
  - Softmax on ACT/DVE; normalized weights split hi/lo and written as fp8
    bytes pair-interleaved into uint16 scatter input.
  - Band build: GPSIMD local_scatter places the 31 taps per (token, head)
    into aligned band blocks (hi band: full 256-sigma window pair-packed;
    lo band: cur-tile 128-sigma window only - prev-tile lo taps are
    dropped, they cover <12% of taps at ~2% weight).
  - Band transpose: ONE chunk-wise DMA transpose (XBAR) per tile replaces
    all PE transposes and the PSUM band evacuation.
  - Conv: per head one DoubleRow matmul with slots (prev tile, cur tile)
    for the hi part of h1 and one for the lo part; h1 lives in a resident
    [128, nt*2048] fp8 hi/lo-interleaved SBUF tensor.
  - Phase D: h2 = conv @ w2.T as 3 compensation terms (ct hi/lo splits on
    the PSUM->SBUF evacuation; w2 hi/lo host-packed).
  - LayerNorm rides the PSUM evacuation: z = h2 + x (bf16), accum sums,
    ACT Square accum for sumsq, rstd = exp(-0.5*ln(var+eps)).
  - Residual x input and final output travel as bf16 (host converts).
"""

import sys

sys.path.insert(0, "/opt/trn_rl_repo")

import numpy as np
from contextlib import ExitStack

import concourse.bass as bass
import concourse.bacc as bacc
import concourse.mybir as mybir
from concourse import tile

T, B, E = 2048, 16, 1024
CDIM, H, KW = 1024, 16, 31
R = CDIM // H            # 64 channels per head
NB = 2                   # batch shard per core
NCORES = 8
P = 128
EPS = 1e-5
BSCALE = 16.0            # band weights scaled by 16 before fp8 quantization

AF = mybir.ActivationFunctionType
ALU = mybir.AluOpType
DR = mybir.MatmulPerfMode.DoubleRow

_ONE_TABLE = "natural_log_exp_and_others"


class _Bacc(bacc.Bacc):
    """Bacc with the ACT table list restricted to one set covering every
    activation function this kernel uses (Exp, Ln, Copy, Square) — the
    default per-activation selection ping-pongs between sets, costing a
    ~1.3us table load per switch."""

    def insert_act_table_loads(self):
        from concourse.hw_specs import get_activation_tables

        has_activation = any(
            isinstance(i, mybir.InstActivation)
            for b in self.main_func.blocks
            for i in b.instructions
        )
        if not has_activation:
            return
        tables = [
            (k, v if k == _ONE_TABLE else set())
            for k, v in get_activation_tables(self.m.arch).items()
        ]
        assert any(v for _, v in tables)
        import bass_rust
        bass_rust.insert_act_table_loads(self, tables)


def _build(t_loc: int, trivial_affine: bool, trivial_bias: bool) -> bacc.Bacc:
    f32 = mybir.dt.float32
    f32r = mybir.dt.float32r
    bf16 = mybir.dt.bfloat16
    fp8 = mybir.dt.float8e4
    u16 = mybir.dt.uint16
    i16 = mybir.dt.int16

    m_loc = NB * t_loc           # tokens per core
    nt = m_loc // P              # token tiles
    tpb = t_loc // P             # tiles per local batch
    tpblk = min(4, nt)           # tiles per x block
    nblk = nt // tpblk

    nc = _Bacc()

    xh_d = nc.dram_tensor("xh", [P, nblk * tpblk * 1024], fp8, kind="ExternalInput")
    xl_d = nc.dram_tensor("xl", [P, nblk * tpblk * 1024], fp8, kind="ExternalInput")
    xtok_d = nc.dram_tensor("xtok", [m_loc, E], bf16, kind="ExternalInput")
    w1h_d = nc.dram_tensor("w1h", [P, 8 * CDIM], fp8, kind="ExternalInput")
    wfh_d = nc.dram_tensor("wfh", [P, 8 * H * KW], fp8, kind="ExternalInput")
    wfl_d = nc.dram_tensor("wfl", [P, 8 * H * KW], fp8, kind="ExternalInput")
    w2h_d = nc.dram_tensor("w2h", [P, 8 * E], fp8, kind="ExternalInput")
    w2l_d = nc.dram_tensor("w2l", [P, 8 * E], fp8, kind="ExternalInput")
    idx_d = nc.dram_tensor("idx", [P, 4 * KW], i16, kind="ExternalInput")
    idx0_d = nc.dram_tensor("idx0", [P, 4 * KW], i16, kind="ExternalInput")
    idxl_d = nc.dram_tensor("idxl", [P, 8 * KW], i16, kind="ExternalInput")
    if not trivial_bias:
        b1r_d = nc.dram_tensor("b1r", [1, CDIM], f32r, kind="ExternalInput")
        bwr_d = nc.dram_tensor("bwr", [1, H * KW], f32r, kind="ExternalInput")
        b2r_d = nc.dram_tensor("b2r", [1, E], f32r, kind="ExternalInput")
        ones_d = nc.dram_tensor("ones", [1, P], f32r, kind="ExternalInput")
    if not trivial_affine:
        gam_d = nc.dram_tensor("gamma_bc", [P, E], bf16, kind="ExternalInput")
        bet_d = nc.dram_tensor("beta_bc", [P, E], bf16, kind="ExternalInput")
    out_d = nc.dram_tensor("out", [m_loc, E], bf16, kind="ExternalOutput")

    with tile.TileContext(nc) as tc, ExitStack() as ctx:
        const = ctx.enter_context(tc.tile_pool(name="const", bufs=1))
        xb_p = ctx.enter_context(tc.tile_pool(name="xb", bufs=2))
        xtk_p = ctx.enter_context(tc.tile_pool(name="xtk", bufs=5))
        sm_p = ctx.enter_context(tc.tile_pool(name="sm", bufs=3))
        bu_p = ctx.enter_context(tc.tile_pool(name="bu", bufs=3))
        bt_p = ctx.enter_context(tc.tile_pool(name="bt", bufs=3))
        ct_p = ctx.enter_context(tc.tile_pool(name="ct", bufs=2))
        z_p = ctx.enter_context(tc.tile_pool(name="z", bufs=2))
        out_p = ctx.enter_context(tc.tile_pool(name="outp", bufs=2))
        ps_a = ctx.enter_context(tc.tile_pool(name="psa", bufs=2, space="PSUM"))
        ps_b = ctx.enter_context(tc.tile_pool(name="psb", bufs=2, space="PSUM"))
        ps_c = ctx.enter_context(tc.tile_pool(name="psc", bufs=2, space="PSUM"))
        ps_d = ctx.enter_context(tc.tile_pool(name="psd", bufs=2, space="PSUM"))

        # resident constants. DMA order: first tile's matmuls need x block 0
        # and w1/wf hi+lo; w2 is only needed ~2 tiles in, so it goes last.
        xt0h = xb_p.tile([P, tpblk * 1024], fp8, tag="xh", name="xt0h")
        xt0l = xb_p.tile([P, tpblk * 1024], fp8, tag="xl", name="xt0l")
        w1h_t = const.tile([P, 8 * CDIM], fp8, tag="w1h")
        wfh_t = const.tile([P, 8 * H * KW], fp8, tag="wfh")
        wfl_t = const.tile([P, 8 * H * KW], fp8, tag="wfl")
        # startup order matches tile 0's matmul emission: x hi + term1
        # weights chunk-by-chunk, x lo midway, then the lo-weight chunks
        cw, cf = 2 * CDIM, 2 * H * KW
        bw0 = tpblk * 1024
        nc.sync.dma_start(xt0h[:, 0:bw0 // 2], xh_d[:, 0:bw0 // 2])
        for q4 in range(4):
            nc.sync.dma_start(w1h_t[:, q4 * cw:(q4 + 1) * cw],
                              w1h_d[:, q4 * cw:(q4 + 1) * cw])
            nc.sync.dma_start(wfh_t[:, q4 * cf:(q4 + 1) * cf],
                              wfh_d[:, q4 * cf:(q4 + 1) * cf])
            if q4 == 1:
                nc.sync.dma_start(xt0h[:, bw0 // 2:bw0],
                                  xh_d[:, bw0 // 2:bw0])
                nc.sync.dma_start(xt0l[:, 0:bw0 // 2], xl_d[:, 0:bw0 // 2])
        nc.sync.dma_start(xt0l[:, bw0 // 2:bw0], xl_d[:, bw0 // 2:bw0])
        for q4 in range(4):
            nc.sync.dma_start(wfl_t[:, q4 * cf:(q4 + 1) * cf],
                              wfl_d[:, q4 * cf:(q4 + 1) * cf])
        idx_t = const.tile([P, 4 * KW], i16, tag="idx")
        nc.sync.dma_start(idx_t[:], idx_d[:])
        idx0_t = const.tile([P, 4 * KW], i16, tag="idx0")
        nc.sync.dma_start(idx0_t[:], idx0_d[:])
        idxl_t = const.tile([P, 8 * KW], i16, tag="idxl")
        nc.sync.dma_start(idxl_t[:], idxl_d[:])
        w2h_t = const.tile([P, 8 * E], fp8, tag="w2h")
        w2l_t = const.tile([P, 8 * E], fp8, tag="w2l")
        nc.sync.dma_start(w2h_t[:], w2h_d[:])
        nc.sync.dma_start(w2l_t[:], w2l_d[:])
        eps_t = const.tile([P, 1], f32, tag="eps")
        nc.vector.memset(eps_t[:], EPS)
        if not trivial_bias:
            b1r = const.tile([1, CDIM], f32r, tag="b1r")
            bwr = const.tile([1, H * KW], f32r, tag="bwr")
            b2r = const.tile([1, E], f32r, tag="b2r")
            ones = const.tile([1, P], f32r, tag="ones")
            nc.sync.dma_start(b1r[:], b1r_d[:])
            nc.sync.dma_start(bwr[:], bwr_d[:])
            nc.sync.dma_start(b2r[:], b2r_d[:])
            nc.sync.dma_start(ones[:], ones_d[:])
        if not trivial_affine:
            gam_t = const.tile([P, E], bf16, tag="gam")
            bet_t = const.tile([P, E], bf16, tag="bet")
            nc.sync.dma_start(gam_t[:], gam_d[:])
            nc.sync.dma_start(bet_t[:], bet_d[:])

        # resident h1 (hi/lo interleaved per tile): [p, (slot, part, c)] with
        # slot t+1 holding tile t; slot 0 is a zeroed dummy so the conv's
        # prev/cur DR pairing is uniform even at the causal start (the band's
        # prev half is all-zero there via the idx0 scatter table).
        h1big = const.tile([P, (nt + 1) * 2048], fp8, tag="h1big")
        nc.vector.memset(h1big[:, 0:2048], 0)
        hb = h1big[:].rearrange(
            "p (t part h m) -> p t part h m", t=nt + 1, part=2, h=H
        )

        w1v = {0: w1h_t[:].rearrange("p (q i n) -> p q i n", q=4, i=2)}
        wfv = {0: wfh_t[:].rearrange("p (q i n) -> p q i n", q=4, i=2),
               1: wfl_t[:].rearrange("p (q i n) -> p q i n", q=4, i=2)}
        w2v = {0: w2h_t[:].rearrange("p (q i n) -> p q i n", q=4, i=2),
               1: w2l_t[:].rearrange("p (q i n) -> p q i n", q=4, i=2)}

        xt = (xt0h, xt0l)
        states: dict = {}     # tile -> (bt_tile, xtok_tile)

        def emit_ab(i, xth, xtl):
            j = i % tpblk
            xvh = xth[:].rearrange("p (pair i t) -> p pair i t", pair=4, i=2)
            xvl = xtl[:].rearrange("p (pair i t) -> p pair i t", pair=4, i=2)
            pa = [ps_a.tile([P, 512], f32, tag="pa", name=f"pa{eb}")
                  for eb in range(2)]
            pb = ps_b.tile([P, H * KW], f32, tag="pb")
            js = slice(j * P, (j + 1) * P)
            # compensation terms (lhs source, weight part); h1 skips the
            # lo-weight term (band-split covers the margin), logits keep it
            terms = [(xvh, 0), (xvl, 0), (xvh, 1)]
            for ti, (xv, wp) in enumerate(terms):
                for q in range(4):
                    lhsT = xv[:, q, :, js]
                    if ti < 2:
                        for half in range(4):
                            n0 = half * 256
                            first = ti == 0 and q == 0 and half % 2 == 0
                            last = (ti == 1 and q == 3 and half % 2 == 1
                                    and trivial_bias)
                            nc.tensor.matmul(
                                pa[half // 2][:, (n0 % 512):(n0 % 512) + 256],
                                lhsT, w1v[wp][:, q, :, n0:n0 + 256],
                                start=first, stop=last, perf_mode=DR,
                                skip_group_check=True,
                            )
                    for half in range(2):
                        n0 = half * 248
                        nc.tensor.matmul(
                            pb[:, n0:n0 + 248],
                            lhsT, wfv[wp][:, q, :, n0:n0 + 248],
                            start=(ti == 0 and q == 0 and half == 0),
                            stop=(ti == 2 and q == 3 and half == 1
                                  and trivial_bias),
                            perf_mode=DR, skip_group_check=True,
                        )
            if not trivial_bias:
                for eb in range(2):
                    nc.tensor.matmul(pa[eb][:], ones[:],
                                     b1r[:, eb * 512:(eb + 1) * 512],
                                     start=False, stop=True)
                nc.tensor.matmul(pb[:], ones[:], bwr[:], start=False, stop=True)
            return pa, pb

        def emit_conv(c, btc):
            """conv for tile c using band bt_c; h1 (moving) from h1big.

            Flipped orientation: the band is the stationary operand so the
            output is [128 tau partitions, 64] at partition base 0 (DoubleRow
            destinations must start at partition 0). Head h -> bank h//8,
            cols (h%8)*64. Slot pairing = (prev tile, cur tile); at c%tpb==0
            the band's prev half is zero (idx0) and slot0 reads the dummy/
            stale h1 slot, contributing exactly zero.

            3 compensation terms per head: bandh*h1h and bandh*h1l as
            DoubleRow over (prev, cur) slots; bandl*h1h as a single
            cur-tile matmul (the lo band only materializes cur taps)."""
            bf8 = btc[:].bitcast(fp8)
            bhi = bf8[:, 0:4096].rearrange(
                "p (j i t b) -> p j i t b", j=8, i=2, t=P, b=2
            )
            blo = bf8[:, 4096:6144].rearrange(
                "p (j t b) -> p j t b", j=8, t=P, b=2
            )
            pc = [ps_c.tile([P, 512], f32, tag="pc", name=f"pc{g}")
                  for g in range(2)]
            for h in range(H):
                g2, hl = h // 8, h % 8
                pair, byte = h // 2, h % 2
                cs = slice(hl * 64, (hl + 1) * 64)
                for part in range(2):
                    nc.tensor.matmul(
                        pc[g2][:, cs],
                        bhi[:, pair, :, :, byte],       # [p, 2, 128] stationary
                        hb[:, c:c + 2, part, h, :],     # [p, 2, 64] moving
                        start=(hl == 0 and part == 0), stop=False,
                        perf_mode=DR, skip_group_check=True,
                    )
                nc.tensor.matmul(
                    pc[g2][:, cs],
                    blo[:, pair, :, byte],              # [p, 128] stationary
                    hb[:, c + 1, 0, h, :],              # [p, 64] moving
                    start=False, stop=True,
                    skip_group_check=True,
                )
            return pc

        def emit_ct_evac(pc):
            """PSUM conv [tau, c] -> SBUF uint16 ct (bytes = cth, ctl),
            un-scaling the x16 band factor; then one chunk-wise DMA
            transpose to get c on partitions for phase D."""
            ct = ct_p.tile([P, 1024], u16, tag="ct")
            ctf = ct[:].bitcast(fp8).rearrange("p (c b) -> p c b", b=2)
            for g2 in range(2):
                cs = slice(g2 * 512, (g2 + 1) * 512)
                nc.scalar.activation(
                    ctf[:, cs, 0], pc[g2][:], AF.Copy, scale=1.0 / BSCALE,
                )
                nc.vector.scalar_tensor_tensor(
                    ctf[:, cs, 1], pc[g2][:], 1.0 / BSCALE, ctf[:, cs, 0],
                    op0=ALU.mult, op1=ALU.subtract,
                )
            ctT = ct_p.tile([P, 1024], u16, tag="ctT")
            nc.sync.dma_start(
                ctT[:].rearrange("p (j b) -> p j b", j=8),
                ct[:], transpose=True,
            )
            return ctT

        def emit_d(ctT):
            ctTf = ctT[:].bitcast(fp8).rearrange(
                "p (ch t b) -> p ch t b", ch=8, t=P, b=2
            )
            pd = [ps_d.tile([P, 512], f32, tag="pd", name=f"pd{eb}")
                  for eb in range(2)]
            # terms: (ct part byte, w2 part)
            terms = [(0, 0), (1, 0), (0, 1)]
            for ti, (cpart, wp) in enumerate(terms):
                for q in range(4):
                    lhsT = ctTf[:, 2 * q:2 * q + 2, :, cpart]
                    for half in range(4):
                        n0 = half * 256
                        first = ti == 0 and q == 0 and half % 2 == 0
                        last = (ti == 2 and q == 3 and half % 2 == 1
                                and trivial_bias)
                        nc.tensor.matmul(
                            pd[half // 2][:, (n0 % 512):(n0 % 512) + 256],
                            lhsT, w2v[wp][:, q, :, n0:n0 + 256],
                            start=first, stop=last, perf_mode=DR,
                            skip_group_check=True,
                        )
            if not trivial_bias:
                for eb in range(2):
                    nc.tensor.matmul(pd[eb][:], ones[:],
                                     b2r[:, eb * 512:(eb + 1) * 512],
                                     start=False, stop=True)
            return pd

        def emit_tail(c, pd, xtok_t):
            zsb = z_p.tile([P, E], bf16, tag="zsb")
            sq = z_p.tile([P, E], bf16, tag="sq")
            st = sm_p.tile([P, 8], f32, tag="st")
            for eb in range(2):
                es = slice(eb * 512, (eb + 1) * 512)
                nc.vector.scalar_tensor_tensor(
                    zsb[:, es], pd[eb][:], 0.0, xtok_t[:, es],
                    op0=ALU.add, op1=ALU.add, accum_out=st[:, eb:eb + 1],
                )
            nc.scalar.activation(
                sq[:], zsb[:], AF.Square, accum_out=st[:, 4:5],
            )
            nc.vector.tensor_scalar(
                st[:, 2:3], st[:, 0:1], 1.0, st[:, 1:2],
                op0=ALU.mult, op1=ALU.add,
            )  # sum = st0 + st1
            nc.vector.tensor_scalar_mul(st[:, 3:4], st[:, 2:3], -1.0 / E)
            nc.vector.tensor_scalar(
                st[:, 7:8], st[:, 3:4], st[:, 3:4], None, op0=ALU.mult
            )  # m2 = negmean^2
            nc.vector.tensor_scalar(
                st[:, 6:7], st[:, 4:5], 1.0 / E, st[:, 7:8],
                op0=ALU.mult, op1=ALU.subtract,
            )  # var = sumsq/E - m2
            lnv = sm_p.tile([P, 2], f32, tag="lnv")
            nc.scalar.activation(lnv[:, 0:1], st[:, 6:7], AF.Ln,
                                 bias=eps_t[:, 0:1])
            nc.scalar.activation(lnv[:, 1:2], lnv[:, 0:1], AF.Exp, scale=-0.5)
            out_t = out_p.tile([P, E], bf16, tag="outt")
            nc.vector.tensor_scalar(
                out_t[:], zsb[:], st[:, 3:4], lnv[:, 1:2],
                op0=ALU.add, op1=ALU.mult,
            )
            if not trivial_affine:
                nc.vector.tensor_mul(out_t[:], out_t[:], gam_t[:])
                nc.vector.tensor_add(out_t[:], out_t[:], bet_t[:])
            nc.sync.dma_start(out_d[c * P:(c + 1) * P, :], out_t[:])

        cstates: dict = {}    # tile -> ctT tile

        for it in range(nt + 3):
            i = it            # tile whose A/B phase is emitted now
            c = it - 2        # tile whose conv/ct phase is emitted now
            d = it - 3        # tile whose D/tail phase is emitted now

            if i < nt:
                # prefetch the next x block two tiles ahead of first use
                if (i + 2) % tpblk == 0 and 0 < i + 2 < nt:
                    blk = (i + 2) // tpblk
                    xth = xb_p.tile([P, tpblk * 1024], fp8, tag="xh",
                                    name=f"xth{blk}")
                    xtl = xb_p.tile([P, tpblk * 1024], fp8, tag="xl",
                                    name=f"xtl{blk}")
                    bw_ = tpblk * 1024
                    nc.sync.dma_start(xth[:], xh_d[:, blk * bw_:(blk + 1) * bw_])
                    nc.sync.dma_start(xtl[:], xl_d[:, blk * bw_:(blk + 1) * bw_])
                    xt_next = (xth, xtl)
                if i % tpblk == 0 and i > 0:
                    xt = xt_next
                xtok_t = xtk_p.tile([P, E], bf16, tag="xtok")
                nc.sync.dma_start(xtok_t[:], xtok_d[i * P:(i + 1) * P, :])
                pa, pb = emit_ab(i, xt[0], xt[1])
                # exp as early as possible (frees pb)
                expw = sm_p.tile([P, H * KW], f32, tag="expw")
                nc.scalar.activation(expw[:], pb[:], AF.Exp)

            if 0 <= c < nt:
                pc = emit_conv(c, states[c][0])

            if i < nt:
                # h1 evacuation (into slot i+1): hi on ACT, lo on DVE
                for eb in range(2):
                    dsth = hb[:, i + 1, 0, eb * 8:(eb + 1) * 8, :]
                    dstl = hb[:, i + 1, 1, eb * 8:(eb + 1) * 8, :]
                    nc.scalar.copy(dsth, pa[eb][:])
                    nc.vector.tensor_sub(dstl, pa[eb][:], dsth)
                # softmax rest (f32 throughout)
                sums = sm_p.tile([P, H], f32, tag="sums")
                nc.vector.tensor_reduce(
                    sums[:], expw[:].rearrange("p (h k) -> p h k", k=KW),
                    axis=mybir.AxisListType.X, op=ALU.add,
                )
                rs16 = sm_p.tile([P, H], f32, tag="rs16")
                nc.vector.reciprocal(rs16[:], sums[:])
                nc.vector.tensor_scalar_mul(rs16[:], rs16[:], BSCALE)
                # normalized weights (x16) in f32 (on GPSIMD), then hi/lo fp8
                # bytes pair-packed per uint16 element: cols 0-247 = hi pairs,
                # 248-495 = lo pairs
                wt = sm_p.tile([P, H * KW], f32, tag="wt")
                nc.vector.tensor_tensor(
                    wt[:].rearrange("p (h k) -> p h k", h=H),
                    expw[:].rearrange("p (h k) -> p h k", h=H),
                    rs16[:].unsqueeze(2).broadcast_to((P, H, KW)),
                    op=ALU.mult,
                )
                wb = sm_p.tile([P, H * KW], u16, tag="wb")
                wbp = wb[:].bitcast(fp8).rearrange(
                    "p (half j k two) -> p half j k two", half=2, j=8, two=2
                )
                tv = wt[:].rearrange("p (h k) -> p h k", h=H)
                for b_ in range(2):
                    hi = wbp[:, 0, :, :, b_]
                    nc.scalar.activation(hi, tv[:, b_::2, :], AF.Copy)
                    nc.vector.tensor_sub(wbp[:, 1, :, :, b_],
                                         tv[:, b_::2, :], hi)
                # band build: 2 hi scatters (full 256-sigma window) + 1 lo
                # scatter (cur-tile taps only). Causal-start tiles use idx0
                # for hi; the lo table drops prev-tile taps always.
                it_sel = idx0_t if (i % tpb) == 0 else idx_t
                bandu = bu_p.tile([P, 3072], u16, tag="bandu")
                for g in range(2):
                    nc.gpsimd.local_scatter(
                        bandu[:, g * 1024:(g + 1) * 1024],
                        wb[:, g * 124:(g + 1) * 124],
                        it_sel[:, 0:124],
                        channels=P, num_elems=1024, num_idxs=124,
                    )
                nc.gpsimd.local_scatter(
                    bandu[:, 2048:3072],
                    wb[:, 248:496],
                    idxl_t[:],
                    channels=P, num_elems=1024, num_idxs=8 * KW,
                )
                # chunk-wise band transpose on the DMA XBAR
                btc = bt_p.tile([P, 3072], u16, tag="bt")
                nc.sync.dma_start(
                    btc[:].rearrange("p (j b) -> p j b", j=24),
                    bandu[:], transpose=True,
                )
                states[i] = (btc, xtok_t)

            if 0 <= c < nt:
                cstates[c] = emit_ct_evac(pc)

            if d >= 0:
                pd = emit_d(cstates.pop(d))
                emit_tail(d, pd, states.pop(d)[1])

    nc.finalize()
    return nc


def _scatter_idx(causal_start: bool) -> np.ndarray:
    t = np.zeros((P, 4 * KW), np.int16)
    for p in range(P):
        for jl in range(4):
            for k in range(KW):
                sig = p + k + 98
                if causal_start and sig < 128:
                    t[p, jl * KW + k] = -1   # ignored: drops pre-batch taps
                else:
                    t[p, jl * KW + k] = jl * 256 + sig
    return t


def _scatter_idx_lo() -> np.ndarray:
    # lo band: cur-tile taps only, 128-sigma window per head pair
    t = np.zeros((P, 8 * KW), np.int16)
    for p in range(P):
        for j in range(8):
            for k in range(KW):
                sl = p + k - 30
                t[p, j * KW + k] = j * 128 + sl if sl >= 0 else -1
    return t


_CACHE: dict = {}


def _get_nc(t_loc: int, trivial: bool, trivial_bias: bool = True):
    key = (t_loc, trivial, trivial_bias)
    if key not in _CACHE:
        _CACHE[key] = _build(t_loc, trivial, trivial_bias)
    return _CACHE[key]


def kernel(x, w1, b1, ww, bw, w2, b2, gamma, beta):
    import ml_dtypes

    f8 = ml_dtypes.float8_e4m3
    bfd = ml_dtypes.bfloat16

    x = np.asarray(x, np.float32)
    w1 = np.asarray(w1, np.float32)
    b1 = np.asarray(b1, np.float32)
    ww = np.asarray(ww, np.float32)
    bw = np.asarray(bw, np.float32)
    w2 = np.asarray(w2, np.float32)
    b2 = np.asarray(b2, np.float32)
    gamma = np.asarray(gamma, np.float32)
    beta = np.asarray(beta, np.float32)

    t_loc, b_full, e = x.shape
    assert e == E and b_full == B

    trivial = bool(np.all(gamma == 1.0) and np.all(beta == 0.0))
    wf = (ww.astype(np.float64) @ w1.astype(np.float64)).astype(np.float32)
    bwf = (ww.astype(np.float64) @ b1.astype(np.float64)).astype(np.float32) + bw
    trivial_bias = bool(
        np.all(b1 == 0.0) and np.all(bwf == 0.0) and np.all(b2 == 0.0)
    )
    nc = _get_nc(t_loc, trivial, trivial_bias)

    m_loc = NB * t_loc
    nt = m_loc // P
    tpblk = min(4, nt)
    nblk = nt // tpblk

    def split8(a):
        hi = a.astype(f8)
        lo = (a - hi.astype(np.float32)).astype(f8)
        return hi, lo

    def pack_w(wT, ncols):
        # wT [K, n] -> [P, 4, 2, ncols]: [p, q, i, :] = wT[q*256 + i*128 + p]
        out = np.empty((P, 4, 2, ncols), wT.dtype)
        for q in range(4):
            for i in range(2):
                out[:, q, i, :] = wT[q * 256 + i * 128:q * 256 + (i + 1) * 128]
        return out.reshape(P, 8 * ncols)

    w1h8 = w1.astype(f8)          # [CDIM, E]
    wfh8, wfl8 = split8(wf)       # [HK, E]
    w2h8, w2l8 = split8(w2)       # [E, CDIM]

    common = {
        "w1h": pack_w(np.ascontiguousarray(w1h8.T), CDIM),
        "wfh": pack_w(np.ascontiguousarray(wfh8.T), H * KW),
        "wfl": pack_w(np.ascontiguousarray(wfl8.T), H * KW),
        "w2h": pack_w(np.ascontiguousarray(w2h8.T), E),
        "w2l": pack_w(np.ascontiguousarray(w2l8.T), E),
        "idx": _scatter_idx(False),
        "idx0": _scatter_idx(True),
        "idxl": _scatter_idx_lo(),
    }
    if not trivial_bias:
        common["b1r"] = b1[None, :]
        common["bwr"] = bwf[None, :]
        common["b2r"] = b2[None, :]
        common["ones"] = np.ones((1, P), np.float32)
    if not trivial:
        common["gamma_bc"] = np.broadcast_to(gamma, (P, E)).astype(bfd).copy()
        common["beta_bc"] = np.broadcast_to(beta, (P, E)).astype(bfd).copy()

    xh8_all = x.astype(f8)
    xl8_all = (x - xh8_all.astype(np.float32)).astype(f8)

    def pack_x(xs8):
        # xs8 [t_loc, NB, E] -> [P, nblk, 4, 2, 512]:
        # [p, blk, pair, i, t] = xs8T[pair*256 + i*128 + p, blk*512 + t]
        xsT = np.ascontiguousarray(
            xs8.transpose(2, 1, 0)).reshape(E, m_loc)      # [E, m] b-major
        out = np.empty((P, nblk, 4, 2, tpblk * P), xs8.dtype)
        for pair in range(4):
            for i in range(2):
                rows = xsT[pair * 256 + i * 128:pair * 256 + (i + 1) * 128]
                out[:, :, pair, i, :] = rows.reshape(P, nblk, tpblk * P)
        return out.reshape(P, nblk * tpblk * 1024)

    in_maps = []
    for cix in range(NCORES):
        sl = slice(NB * cix, NB * (cix + 1))
        xh8 = xh8_all[:, sl, :]
        xl8 = xl8_all[:, sl, :]
        xtok = np.ascontiguousarray(
            x[:, sl, :].transpose(1, 0, 2)).reshape(m_loc, E).astype(bfd)
        m = dict(common)
        m["xh"] = pack_x(xh8)
        m["xl"] = pack_x(xl8)
        m["xtok"] = xtok
        in_maps.append(m)

    from concourse.bass_utils import run_bass_kernel_spmd

    res = run_bass_kernel_spmd(nc, in_maps, core_ids=list(range(NCORES)))

    out = np.empty((t_loc, B, E), np.float32)
    for cix in range(NCORES):
        oc = np.asarray(res.results[cix]["out"]).view(bfd).astype(
            np.float32).reshape(NB, t_loc, E)
        for bl in range(NB):
            out[:, NB * cix + bl, :] = oc[bl]
    return out
